# revision 31
# baseline (speedup 1.0000x reference)
"""Multi-head attention forward on 8 Trainium2 NeuronCores.

Problem: B=32, N=512, C=1024, H=16 heads, head_dim=64, fp32 I/O.
Strategy: data-parallel over batch (4 batches per core), no collectives.

Math notes:
  - reference adds mask[:,None,None,:] + mask[:,None,:,None] to the logits;
    the query-axis term is constant along the softmax axis so it cancels.
    The key-axis term is folded in as exp(mask)[k], applied by scaling V rows
    and the softmax-denominator column.
  - softmax denominator comes out of the attn@V matmul for free: V is
    augmented with a 65th column holding exp(mask)[k], so
    out[q,64] = sum_k e[q,k]*em[k] = denominator.

Layouts (per core, per batch):
  xT [nt][c,128] (host pre-transposed, n-tile-major so the b=0 V stage can
  start after the first 1 MiB chunk) -> Q^T,K^T [dc,n] via W-stationary
  matmuls, V [n,dc] via x-stationary matmuls (scaled by em on evac).
  scores^T [k,q] per head (contract d=64) into 2-bank psum tiles; even/odd
  heads read qkT partitions 0:64 / 64:128 so bass emits them as 64x128
  row-tiled matmuls on independent PE tiles T0/T8, pairwise interleaved;
  exp on ScalarE over [128,1024] at a time -> e^T bf16.
  attn@V: lhsT=e^T[k,q-tile], rhs=V_aug[k,65] -> psum [q,65]; normalize with
  reciprocal of col 64 (per-partition scalar) -> attn [q,c] bf16.
  PE-transpose attn -> attn^T [c,q]; proj: lhsT=attn^T, rhs=W_proj^T -> out.

Schedule (per batch): V first (fused into x-chunk arrival), then QK pairs
prefetched TWO steps ahead of the scores that consume them (so the DVE
psum->sbuf evacuation of QK is never on the PE critical path). Within a
step the engine-tiling-mode phases are kept contiguous (each 128x128 <->
64x128 mode switch drains the PE array): qk pair, scores half, attn@V,
scores half, attn@V + transpose; the attn@V between the score halves
gives ScalarE time to drain the first half's psum tiles. Projection last.
"""
import numpy as np
import ml_dtypes

B, N, C, H = 32, 512, 1024, 16
HD = C // H  # 64
SCALE = HD ** -0.5
NCORES = 8
BL = B // NCORES  # batches per core = 4
CT = C // 128     # 8 c-tiles
NT = N // 128     # 4 n-tiles
DC3 = 3 * C       # 3072

_cached_nc = None


def _build(repeat=1):
    import contextlib
    import concourse.mybir as mybir
    import concourse.tile as tile
    from concourse import bacc
    from concourse.masks import make_identity

    BF16 = mybir.dt.bfloat16
    F32 = mybir.dt.float32
    EXP = mybir.ActivationFunctionType.Exp

    nc = bacc.Bacc()
    # Host pre-arranges x / weights in SBUF-partition-major layouts so every
    # DMA source run is >=2KB contiguous (sub-512B runs pay a 2x DMA-latency
    # penalty on TRN2).
    xT_d = nc.dram_tensor("xT", [BL, NT, 128, CT, 128], BF16, kind="ExternalInput")
    wqkvT_d = nc.dram_tensor("wqkvT", [24, 128, CT, 128], BF16, kind="ExternalInput")
    wprojT_d = nc.dram_tensor("wprojT", [128, CT, C], BF16, kind="ExternalInput")
    em_d = nc.dram_tensor("em", [128, BL, NT], F32, kind="ExternalInput")
    out_d = nc.dram_tensor("out", [BL, N, C], BF16, kind="ExternalOutput")

    with tile.TileContext(nc) as tc:
        with (
            tc.tile_pool(name="singles", bufs=1) as singles,
            tc.tile_pool(name="xp", bufs=2) as xp,
            tc.tile_pool(name="qkp", bufs=2) as qkp,
            tc.tile_pool(name="vp", bufs=2) as vp,
            tc.tile_pool(name="ep", bufs=6) as ep,
            tc.tile_pool(name="ap", bufs=2) as ap,
            tc.tile_pool(name="atp", bufs=2) as atp,
            tc.tile_pool(name="op", bufs=3) as op,
            tc.tile_pool(name="rp", bufs=8) as rp,
            tc.tile_pool(name="ps_big", bufs=2, space="PSUM") as ps_big,
            tc.tile_pool(name="ps_sc", bufs=2, space="PSUM") as ps_sc,
            tc.tile_pool(name="ps_avtr", bufs=2, space="PSUM") as ps_avtr,
        ):
            # --- one-time loads, ordered to unblock compute ASAP:
            # em, V-weight slices, then b=0 x chunks (n-tile-major) inter-
            # leaved with the remaining V weights, then QK weight slices in
            # first-use order (0,8,1,9,...), wproj last. ---
            em_sb = singles.tile([128, BL, NT], F32)
            nc.sync.dma_start(out=em_sb[:], in_=em_d[:])
            wqkvT_sb = singles.tile([128, CT, DC3], BF16)

            def load_w(j):
                nc.sync.dma_start(
                    out=wqkvT_sb.rearrange("p ct (j d) -> p ct j d", d=128)[:, :, j, :],
                    in_=wqkvT_d[j])

            def load_x(dst, b, nt):
                # x rides the Activation HWDGE queue: overlaps the SP queue's
                # weight loads at startup and keeps later batches' x loads
                # from queuing behind projection-output DMAs.
                nc.scalar.dma_start(
                    out=dst[:, :, nt, :],
                    in_=xT_d[b, nt])

            for j in (16, 17, 18, 19):
                load_w(j)
            xT_first = xp.tile([128, CT, NT, 128], BF16, tag="xT")
            load_x(xT_first, 0, 0)
            for j in (20, 21, 22, 23):
                load_w(j)
            for nt in range(1, NT):
                load_x(xT_first, 0, nt)
            for jj in range(8):
                load_w(jj)
                load_w(8 + jj)
            ident = singles.tile([128, 128], BF16)
            make_identity(nc, ident[:])
            wprojT_sb = singles.tile([128, CT, C], BF16)
            nc.sync.dma_start(out=wprojT_sb[:], in_=wprojT_d[:])

            rep_ctx = tc.For_i(0, repeat, 1) if repeat > 1 else contextlib.nullcontext()
            with rep_ctx:
              for b in range(BL):
                # --- load x^T for this batch (b=0 preloaded above) ---
                if b == 0:
                    xT_sb = xT_first
                else:
                    xT_sb = xp.tile([128, CT, NT, 128], BF16, tag="xT")
                    for nt in range(NT):
                        load_x(xT_sb, b, nt)

                # --- V natural [n, dc_v] FIRST (scaled by em, + aug col) so
                # attn@V can fuse directly into the QK loop below. nt-major
                # so the first psum group only needs the first x chunk. ---
                vaug_sb = vp.tile([128, NT, H, HD + 1], BF16, tag="vaug")
                for nt in range(NT):
                    for dcv in range(2):
                        ps = ps_big.tile([128, 512], F32, tag="big")
                        for ct in range(CT):
                            nc.tensor.matmul(
                                ps[:],
                                xT_sb[:, ct, nt, :],
                                wqkvT_sb[:, ct, 2 * C + dcv * 512:2 * C + (dcv + 1) * 512],
                                start=(ct == 0), stop=(ct == CT - 1))
                        nc.vector.tensor_scalar_mul(
                            vaug_sb[:, nt, 8 * dcv:8 * (dcv + 1), 0:HD],
                            ps.rearrange("p (h d) -> p h d", d=HD),
                            em_sb[:, b, nt:nt + 1])
                    nc.vector.tensor_copy(
                        out=vaug_sb[:, nt, :, HD],
                        in_=em_sb[:, b, nt:nt + 1].to_broadcast((128, H)))

                qkT_sb = qkp.tile([128, 16, N], BF16, tag="qkT")

                def emit_qk(dct):
                    ps = ps_big.tile([128, 512], F32, tag="big")
                    for ct in range(CT):
                        nc.tensor.matmul(
                            ps[:],
                            wqkvT_sb[:, ct, dct * 128:(dct + 1) * 128],
                            xT_sb[:, ct, :, :],
                            start=(ct == 0), stop=(ct == CT - 1))
                    nc.vector.tensor_copy(out=qkT_sb[:, dct, :], in_=ps[:])

                # --- per head: scores^T into 2-bank psum tiles, exp on
                # ScalarE over [128,1024] -> e^T bf16; then attn@V_aug +
                # normalize. ---
                attn_sb = ap.tile([128, NT, C], BF16, tag="attn")
                eT_tiles = {}

                def emit_scores_half(h0, h1, half):
                    # The even head reads qkT partitions 0:64, the odd head
                    # 64:128, so bass emits them as 64x128 row-tiled matmuls
                    # on independent PE tiles T0/T8; interleave the two
                    # heads' instructions pairwise so the tiles overlap.
                    dct_q = h0 // 2
                    dct_k = 8 + h0 // 2
                    eT0, eT1 = eT_tiles[h0], eT_tiles[h1]
                    ps0 = ps_sc.tile([128, 2, 512], F32, tag="sc")
                    ps1 = ps_sc.tile([128, 2, 512], F32, tag="sc")
                    for sub in range(2):
                        kt = half * 2 + sub
                        # 64x64 four-tile mode: each head's k-tile split into
                        # two 64-wide halves landing on psum partition halves;
                        # tiles T0/T2/T8/T10 are emitted back-to-back so the
                        # engine can overlap their streams if capable.
                        for po, ps in ((0, ps0), (HD, ps1)):
                            for kh in range(2):
                                ksl = slice(kt * 128 + kh * 64,
                                            kt * 128 + (kh + 1) * 64)
                                nc.tensor.matmul(
                                    ps[kh * 64:(kh + 1) * 64, sub, :],
                                    qkT_sb[po:po + HD, dct_k, ksl],
                                    qkT_sb[po:po + HD, dct_q, :],
                                    start=True, stop=True)
                    sl = slice(half * 2, half * 2 + 2)
                    nc.scalar.activation(eT0[:, sl, :], ps0[:], EXP, scale=SCALE)
                    nc.scalar.activation(eT1[:, sl, :], ps1[:], EXP, scale=SCALE)

                def emit_attnv(h):
                    eT_sb = eT_tiles.pop(h)
                    for qt in range(NT):
                        psa_t = ps_avtr.tile([128, 128], F32, tag="avtr")
                        psa = psa_t[:, 0:HD + 1]
                        for kt in range(NT):
                            nc.tensor.matmul(
                                psa,
                                eT_sb[:, kt, qt * 128:(qt + 1) * 128],
                                vaug_sb[:, kt, h, :],
                                start=(kt == 0), stop=(kt == NT - 1))
                        recip = rp.tile([128, 1], F32, tag="recip")
                        nc.vector.reciprocal(recip[:], psa_t[:, HD:HD + 1])
                        nc.vector.tensor_scalar_mul(
                            attn_sb[:, qt, h * HD:(h + 1) * HD],
                            psa_t[:, 0:HD], recip[:])

                # transposes of attn c-tile ct are emitted as soon as heads
                # 2ct and 2ct+1 are done, hiding behind later heads' matmuls.
                attnT_sb = atp.tile([128, CT, N], BF16, tag="attnT")

                def emit_transpose(ct):
                    for qt in range(NT):
                        pst = ps_avtr.tile([128, 128], BF16, tag="avtr")
                        nc.tensor.transpose(
                            pst[:], attn_sb[:, qt, ct * 128:(ct + 1) * 128], ident[:])
                        nc.vector.tensor_copy(
                            out=attnT_sb[:, ct, qt * 128:(qt + 1) * 128], in_=pst[:])

                # Fused schedule: QK pairs run TWO steps ahead of the scores
                # that read them (so the DVE qkT evacuation is off the PE
                # critical path); within a step the engine-mode phases are
                # kept contiguous (each 128x128 <-> 64x128 tiling-mode switch
                # drains the PE array): qk, scores-half, attn@V, scores-half,
                # attn@V + transpose. The attn@V between the halves gives
                # ScalarE time to drain the first half's psum tiles.
                emit_qk(0); emit_qk(8)
                emit_qk(1); emit_qk(9)
                for j in range(8):
                    if j < 6:
                        emit_qk(j + 2)
                        emit_qk(10 + j)
                    eT_even = ep.tile([128, NT, N], BF16, tag="eT")
                    eT_odd = ep.tile([128, NT, N], BF16, tag="eT")
                    eT_tiles[2 * j] = eT_even
                    eT_tiles[2 * j + 1] = eT_odd
                    emit_scores_half(2 * j, 2 * j + 1, 0)
                    if j > 0:
                        emit_attnv(2 * j - 2)
                    emit_scores_half(2 * j, 2 * j + 1, 1)
                    if j > 0:
                        emit_attnv(2 * j - 1)
                        emit_transpose(j - 1)
                emit_attnv(H - 2)
                emit_attnv(H - 1)
                emit_transpose(CT - 1)

                # --- projection ---
                for qt in range(NT):
                    out_sb = op.tile([128, C], BF16, tag="out")
                    for cot in range(2):
                        ps = ps_big.tile([128, 512], F32, tag="big")
                        for ct in range(CT):
                            nc.tensor.matmul(
                                ps[:],
                                attnT_sb[:, ct, qt * 128:(qt + 1) * 128],
                                wprojT_sb[:, ct, cot * 512:(cot + 1) * 512],
                                start=(ct == 0), stop=(ct == CT - 1))
                        nc.vector.tensor_copy(out=out_sb[:, cot * 512:(cot + 1) * 512], in_=ps[:])
                        nc.sync.dma_start(
                            out=out_d[b, qt * 128:(qt + 1) * 128,
                                      cot * 512:(cot + 1) * 512],
                            in_=out_sb[:, cot * 512:(cot + 1) * 512])
    nc.finalize()
    return nc


def _prep_inputs(x, mask, W_qkv, W_proj):
    bf16 = ml_dtypes.bfloat16
    # x^T chunks in SBUF layout [B, NT, 128(p), CT, 128(n)] with c=ct*128+p
    xT = np.ascontiguousarray(
        x.transpose(0, 2, 1).reshape(B, CT, 128, NT, 128).transpose(0, 3, 2, 1, 4)
    ).astype(bf16)
    # W_qkv^T as 24 contiguous chunks [j, 128(p), CT, 128(d)] with d=j*128+dd
    wqkvT = np.ascontiguousarray(
        W_qkv.T.reshape(CT, 128, 24, 128).transpose(2, 1, 0, 3)).astype(bf16)
    # W_proj^T in SBUF layout [128(p), CT, C]
    wprojT = np.ascontiguousarray(
        W_proj.T.reshape(CT, 128, C).transpose(1, 0, 2)).astype(bf16)
    em = np.exp(mask).astype(np.float32)                              # [B, N]
    # pre-striped for SBUF layout [p, b_local, nt]: em[b, nt*128+p]
    em_striped = np.ascontiguousarray(
        em.reshape(B, NT, 128).transpose(2, 0, 1))        # [128, B, NT]
    return [
        {
            "xT": xT[c * BL:(c + 1) * BL],
            "wqkvT": wqkvT,
            "wprojT": wprojT,
            "em": em_striped[:, c * BL:(c + 1) * BL, :],
        }
        for c in range(NCORES)
    ]


def build_null():
    """Null kernel with the same external I/O — timing baseline."""
    import concourse.mybir as mybir
    import concourse.tile as tile
    from concourse import bacc

    BF16 = mybir.dt.bfloat16
    F32 = mybir.dt.float32
    nc = bacc.Bacc()
    nc.dram_tensor("xT", [BL, NT, 128, CT, 128], BF16, kind="ExternalInput")
    nc.dram_tensor("wqkvT", [24, 128, CT, 128], BF16, kind="ExternalInput")
    nc.dram_tensor("wprojT", [128, CT, C], BF16, kind="ExternalInput")
    em_d = nc.dram_tensor("em", [128, BL, NT], F32, kind="ExternalInput")
    out_d = nc.dram_tensor("out", [BL, N, C], BF16, kind="ExternalOutput")
    with tile.TileContext(nc) as tc:
        with tc.tile_pool(name="sb", bufs=1) as sb:
            t = sb.tile([128, 4], F32)
            nc.sync.dma_start(out=t[:], in_=em_d[:, 0, :])
            t2 = sb.tile([128, 4], BF16)
            nc.vector.tensor_copy(out=t2[:], in_=t[:])
            nc.sync.dma_start(out=out_d[0, 0:128, 0:4], in_=t2[:])
    nc.finalize()
    return nc


def get_nc():
    global _cached_nc
    if _cached_nc is None:
        _cached_nc = _build()
    return _cached_nc


def kernel(x, mask, W_qkv, W_proj, b_proj):
    from concourse.bass_utils import run_bass_kernel_spmd

    nc = get_nc()
    in_maps = _prep_inputs(np.asarray(x, dtype=np.float32),
                           np.asarray(mask, dtype=np.float32),
                           np.asarray(W_qkv, dtype=np.float32),
                           np.asarray(W_proj, dtype=np.float32))
    res = run_bass_kernel_spmd(nc, in_maps, core_ids=list(range(NCORES)))
    out = np.concatenate([res.results[c]["out"] for c in range(NCORES)], axis=0)
    out = out + np.asarray(b_proj, dtype=np.float32)[None, None, :]
    return np.ascontiguousarray(out.astype(np.float32))
